# revision 15
# baseline (speedup 1.0000x reference)
"""Causal dot-product attention (B=4, H=16, S=2048, D=128) on 8 TRN2 NeuronCores.

Sharding: batch*heads = 64 (b,h) pairs -> 8 heads per core (head parallel, no
communication). Each core runs a flash-attention-style kernel:

  - Host pre-transposes Q,K per head to [D=128, S] in BF16 and packs V per
    head as [kpos=128, kblock, D+1] in bf16 with a ones column at d=128 (the
    PV matmul then produces the softmax denominator for free).
  - Device computes S^T chunks st[k, q] = K_j^T.T @ Q_tile (bf16 matmuls,
    fp32 PSUM), exp(scale*st) on the scalar engine over ~1536-col groups
    (PSUM -> bf16 SBUF), triangular-mask multiplies on diagonal chunks
    (DVE), then PV: acc[q, 0:129] += pt_chunk.T @ V_aug in bf16, accumulated
    over k-blocks in a PSUM bank.
  - Q-tiles are processed in PAIRS with the odd tile's diagonal chunk
    emitted first, so the two 128-wide diagonal chunks sit adjacent in the
    score stream: every 256-wide chunk stays 1KB-aligned inside a PSUM bank
    while the fully-masked half of the diagonal is never computed or exp'd
    (block-causal exact: 17408 of 18432 cols per head).
  - PV/mask/normalize work for group g is emitted interleaved with the QK
    chunks of group g+2: the PE sees QK-MM / PV-MM / PV-MM triples, so every
    LDWEIGHTS hides under a streaming matmul and the in-order PE queue never
    head-of-line blocks on the in-flight exp().
  - Normalize: out = acc[:, :128] * reciprocal(acc[:, 128]) on DVE, written
    bf16 into a per-head staging tile; two output DMAs per head (GpSimd
    queue). Host un-transposes and converts to fp32.

No max-subtraction is needed: scores are ~N(0,1) after the 1/sqrt(128)
scale, so exp() stays in [e-6, e+6] comfortably inside fp32/bf16 range.
"""

import math
import sys
from collections import deque
from contextlib import ExitStack

import numpy as np

for _p in ("/opt/trn_rl_repo", "/root/.axon_site/_ro/trn_rl_repo"):
    if _p not in sys.path:
        sys.path.append(_p)

import ml_dtypes

import concourse.bass as bass
import concourse.tile as tile
from concourse import bacc, mybir
from concourse.bass_utils import run_bass_kernel_spmd

F32 = mybir.dt.float32
BF16 = mybir.dt.bfloat16
AF = mybir.ActivationFunctionType

# Problem constants (hardcoded; kernel.py must be self-contained).
B, H, S, D = 4, 16, 2048, 128
P = 128
N_CORES = 8
NH = (B * H) // N_CORES  # heads per core = 8
SCALE = 1.0 / math.sqrt(128.0)  # D_MODEL = 128

QTW = 256   # q-tile width (two 128-row PSUM subtiles per tile)
GCOLS = 6 * 256  # score-group capacity per exp() (st tile = 3 PSUM banks)
LAG = 2     # groups of deferral between exp() and its PV consumption
DRAIN_K = 4  # deferred-work items emitted after each QK chunk


def build_nc(nh=NH, s=S):
    nkb = s // P   # k-blocks per head = 16
    nqt = s // QTW  # q-tiles per head = 8

    nc = bacc.Bacc("TRN2", target_bir_lowering=False, debug=False,
                   enable_asserts=False)
    qt_d = nc.declare_dram_parameter("qt", [nh, P, s], BF16, isOutput=False).ap()
    kt_d = nc.declare_dram_parameter("kt", [nh, P, s], BF16, isOutput=False).ap()
    v_d = nc.declare_dram_parameter("v", [nh, P, nkb, D + 1], BF16,
                                    isOutput=False).ap()
    # mask = [triu | ones | triu | triu]: col 0:128 is the plain triangular
    # mask; the full 512 pattern covers the (a-last-256, a-diag, b-diag)
    # chunk triple in one DVE multiply.
    mask_d = nc.declare_dram_parameter("mask", [P, 4 * P], BF16,
                                       isOutput=False).ap()
    # Output layout [head, qpos-within-subtile, q-subtile, d] so each head is
    # two DMAs with 2-4 KiB contiguous runs per partition; host un-transposes.
    out_d = nc.declare_dram_parameter("out", [nh, P, nkb, D], BF16,
                                      isOutput=True).ap()

    with tile.TileContext(nc) as tc, ExitStack() as ctx:
        kt_pool = ctx.enter_context(tc.tile_pool(name="kt_pool", bufs=3))
        qt_pool = ctx.enter_context(tc.tile_pool(name="qt_pool", bufs=3))
        v_pool = ctx.enter_context(tc.tile_pool(name="v_pool", bufs=3))
        pt_pool = ctx.enter_context(tc.tile_pool(name="pt_pool", bufs=6))
        stage_pool = ctx.enter_context(tc.tile_pool(name="stage_pool", bufs=3))
        rl_pool = ctx.enter_context(tc.tile_pool(name="rl_pool", bufs=8))
        st_pool = ctx.enter_context(tc.tile_pool(name="st_pool", bufs=2,
                                                 space="PSUM"))
        acc_pool = ctx.enter_context(tc.tile_pool(name="acc_pool", bufs=2,
                                                  space="PSUM"))
        misc = ctx.enter_context(tc.tile_pool(name="misc", bufs=1))

        # Warm the ACT exp table immediately (one-time ~2.7us load) so the
        # first real exp() isn't delayed; operand values are don't-care.
        warm_t = misc.tile([P, 1], F32)
        nc.vector.memset(warm_t[:], 0.0)
        nc.scalar.activation(warm_t[:], warm_t[:], AF.Exp, bias=0.0, scale=1.0)

        mask_t = misc.tile([P, 4 * P], BF16)

        state = {"st": None, "pt": None, "fill": 0, "entries": []}
        pend = deque()      # item-groups not yet eligible (LAG deep)
        eligible = deque()  # flat deferred-work queue, drained between QK MMs

        def emit_item(it):
            kind = it[0]
            if kind == "mask":
                _, pt_t, lo, w, mlo = it
                nc.vector.tensor_mul(pt_t[:, lo:lo + w], pt_t[:, lo:lo + w],
                                     mask_t[:, mlo:mlo + w])
            elif kind == "pv":
                _, pt_t, pos, sI, i, j, acc_t, v_t, st_f, sp_f = it
                nc.tensor.matmul(acc_t[:, sI * 129:(sI + 1) * 129],
                                 lhsT=pt_t[:, pos:pos + P],
                                 rhs=v_t[:, j], start=st_f, stop=sp_f)
            elif kind == "norm":
                _, i, acc_t, stage_t = it
                for sI in range(2):
                    g = 2 * i + sI
                    rl = rl_pool.tile([P, 1], F32, tag="rl", name="rl")
                    nc.vector.reciprocal(
                        rl[:], acc_t[:, sI * 129 + 128:sI * 129 + 129])
                    nc.vector.tensor_scalar_mul(
                        stage_t[:, g * P:(g + 1) * P],
                        acc_t[:, sI * 129:sI * 129 + D], rl[:])
            elif kind == "outdma":
                # Sync-queue HWDGE: much faster than the GpSimd SWDGE path,
                # which costs a ~3.6us drain at the end of the kernel.
                _, h, stage_t, lo, hi = it
                nc.sync.dma_start(out=out_d[h, :, lo:hi],
                                  in_=stage_t[:, lo * D:hi * D])

        def drain(k):
            while k > 0 and eligible:
                emit_item(eligible.popleft())
                k -= 1

        def flush(final=False):
            if state["fill"]:
                w = state["fill"]
                st_t, pt_t = state["st"], state["pt"]
                nc.scalar.activation(pt_t[:, :w], st_t[:, :w], AF.Exp,
                                     bias=0.0, scale=SCALE)
                items = []
                ents = state["entries"]
                skip_mask = set()
                for idx, (pos, wd, eh, i, j, acc_t, v_t, stage_t) in \
                        enumerate(ents):
                    even = (i % 2 == 0)
                    diag128 = (wd == P)  # j == 2i+1, subtile 1 only
                    # --- triangular masks ---
                    if j == 2 * i and idx not in skip_mask:
                        # last full chunk of tile i: triangle on subtile 0.
                        # Merge with an immediately following a-diag/b-diag
                        # pair (positions pos+256, pos+384) when present.
                        if (idx + 2 < len(ents) and ents[idx + 1][1] == P
                                and ents[idx + 2][1] == P
                                and ents[idx + 1][0] == pos + QTW
                                and ents[idx + 2][0] == pos + QTW + P):
                            items.append(("mask", pt_t, pos, 4 * P, 0))
                            skip_mask.add(idx + 1)
                            skip_mask.add(idx + 2)
                        else:
                            items.append(("mask", pt_t, pos, P, 0))
                    elif diag128 and idx not in skip_mask:
                        items.append(("mask", pt_t, pos, P, 0))
                    # --- PV matmuls ---
                    if diag128:
                        st_f = not even          # odd tile: emitted first
                        sp_f = even              # even tile: emitted last
                        items.append(("pv", pt_t, pos, 1, i, j, acc_t, v_t,
                                      st_f, sp_f))
                        if even:
                            items.append(("norm", i, acc_t, stage_t))
                    else:
                        for sI in range(2):
                            st_f = even and j == 0 and sI == 0
                            sp_f = (not even) and j == 2 * i and sI == 1
                            items.append(("pv", pt_t, pos + sI * P, sI, i, j,
                                          acc_t, v_t, st_f, sp_f))
                        if (not even) and j == 2 * i:
                            items.append(("norm", i, acc_t, stage_t))
                    if items and items[-1][0] == "norm":
                        # stream the output out as subtiles complete; the
                        # final piece is tiny so the kernel tail is short.
                        if i == 3:
                            items.append(("outdma", eh, stage_t, 0, 8))
                        elif i == 6:
                            items.append(("outdma", eh, stage_t, 8, 14))
                        elif i == 7:
                            items.append(("outdma", eh, stage_t, 14, nkb))
                pend.append(items)
                lag = 1 if state.get("taillag") else LAG
                while len(pend) > lag:
                    eligible.extend(pend.popleft())
                state.update(st=None, pt=None, fill=0, entries=[])
            if final:
                while pend:
                    eligible.extend(pend.popleft())
                while eligible:
                    emit_item(eligible.popleft())

        stash = {}

        def start_head(h, fine=False):
            kt_t = kt_pool.tile([P, s], BF16, tag="kt", name="kt_t")
            qt_t = qt_pool.tile([P, s], BF16, tag="qt", name="qt_t")
            v_t = v_pool.tile([P, nkb, D + 1], BF16, tag="v", name="v_t")
            stash[h] = (kt_t, qt_t, v_t)
            if fine:
                # head 0: fine-grained loads split across two idle DMA issue
                # queues (kt on Sync, qt/v on GpSimd) so the per-partition
                # descriptor processing of the first chunks runs in parallel
                # and the first matmuls start sooner.
                nc.sync.dma_start(out=kt_t[:, 0:256], in_=kt_d[h, :, 0:256])
                nc.gpsimd.dma_start(out=qt_t[:, 0:256], in_=qt_d[h, :, 0:256])
                nc.sync.dma_start(out=kt_t[:, 256:512],
                                  in_=kt_d[h, :, 256:512])
                nc.gpsimd.dma_start(out=qt_t[:, 256:512],
                                    in_=qt_d[h, :, 256:512])
                nc.gpsimd.dma_start(out=v_t[:, :4], in_=v_d[h, :, :4])
                nc.sync.dma_start(out=mask_t[:], in_=mask_d)
                nc.sync.dma_start(out=kt_t[:, 512:1024],
                                  in_=kt_d[h, :, 512:1024])
                nc.gpsimd.dma_start(out=qt_t[:, 512:s], in_=qt_d[h, :, 512:s])
                nc.sync.dma_start(out=kt_t[:, 1024:s], in_=kt_d[h, :, 1024:s])
                nc.gpsimd.dma_start(out=v_t[:, 4:], in_=v_d[h, :, 4:])
            else:
                nc.sync.dma_start(out=kt_t[:], in_=kt_d[h])
                nc.gpsimd.dma_start(out=qt_t[:], in_=qt_d[h])
                nc.gpsimd.dma_start(out=v_t[:], in_=v_d[h])

        def emit_chunk(h, i, j, width, kt_t, qt_t, v_t, acc_t, stage_t):
            if state["fill"] + width > GCOLS or \
                    (width == QTW and state["fill"] % 512 == 384):
                flush()
            if state["fill"] == 0:
                state["st"] = st_pool.tile([P, GCOLS], F32, tag="st",
                                           name="st_t")
                state["pt"] = pt_pool.tile([P, GCOLS], BF16, tag="pt",
                                           name="pt_t")
            pos = state["fill"]
            qlo = i * QTW if width == QTW else i * QTW + P
            nc.tensor.matmul(state["st"][:, pos:pos + width],
                             lhsT=kt_t[:, j * P:(j + 1) * P],
                             rhs=qt_t[:, qlo:qlo + width],
                             start=True, stop=True)
            state["entries"].append((pos, width, h, i, j, acc_t, v_t, stage_t))
            state["fill"] += width
            if state["fill"] == GCOLS:
                flush()
            drain(DRAIN_K)

        start_head(0, fine=True)
        if nh > 1:
            start_head(1)
        for h in range(nh):
            kt_t, qt_t, v_t = stash.pop(h)
            stage_t = stage_pool.tile([P, nkb * D], BF16, tag="stage",
                                      name="stage_t")
            for a in range(0, nqt, 2):  # q-tile pairs (a, a+1)
                b = a + 1
                if a == nqt - 2 and h + 2 < nh:
                    start_head(h + 2)
                if a == nqt - 2 and h == nh - 1:
                    state["taillag"] = True  # drain eagerly near the end
                acc_a = acc_pool.tile([P, 512], F32, tag="acc", name="acc_a")
                for j in range(2 * a + 1):
                    emit_chunk(h, a, j, QTW, kt_t, qt_t, v_t, acc_a, stage_t)
                emit_chunk(h, a, 2 * a + 1, P, kt_t, qt_t, v_t, acc_a, stage_t)
                acc_b = acc_pool.tile([P, 512], F32, tag="acc", name="acc_b")
                emit_chunk(h, b, 2 * b + 1, P, kt_t, qt_t, v_t, acc_b, stage_t)
                for j in range(2 * b + 1):
                    emit_chunk(h, b, j, QTW, kt_t, qt_t, v_t, acc_b, stage_t)
        flush(final=True)
    nc.compile()
    return nc


_NC = None


def _get_nc():
    global _NC
    if _NC is None:
        _NC = build_nc()
    return _NC


def prepare_in_maps(Q, K, V):
    """Shard + lay out full [B,H,S,D] inputs into per-core in_maps."""
    Qf = np.asarray(Q, dtype=np.float32).reshape(B * H, S, D)
    Kf = np.asarray(K, dtype=np.float32).reshape(B * H, S, D)
    Vf = np.asarray(V, dtype=np.float32).reshape(B * H, S, D)
    nkb = S // P
    tri = np.triu(np.ones((P, P), dtype=np.float32))
    ones = np.ones((P, P), dtype=np.float32)
    mask = np.concatenate([tri, ones, tri, tri], axis=1).astype(
        ml_dtypes.bfloat16)
    in_maps = []
    for c in range(N_CORES):
        hs = slice(c * NH, (c + 1) * NH)
        qt = np.ascontiguousarray(
            Qf[hs].transpose(0, 2, 1)).astype(ml_dtypes.bfloat16)
        kt = np.ascontiguousarray(
            Kf[hs].transpose(0, 2, 1)).astype(ml_dtypes.bfloat16)
        # V: [NH, S, D] -> [NH, kblock, kpos, D] -> [NH, kpos, kblock, D]
        vv = Vf[hs].reshape(NH, nkb, P, D).transpose(0, 2, 1, 3)
        v_aug = np.ones((NH, P, nkb, D + 1), dtype=ml_dtypes.bfloat16)
        v_aug[..., :D] = vv.astype(ml_dtypes.bfloat16)
        in_maps.append({"qt": qt, "kt": kt, "v": v_aug, "mask": mask})
    return in_maps


def gather_out(results):
    # out per core: [NH, P, nkb, D] bf16, q = subtile*128 + p
    outs = []
    for r in results:
        o = np.asarray(r["out"]).astype(np.float32)  # [NH, 128, 16, 128]
        outs.append(o.transpose(0, 2, 1, 3).reshape(NH, S, D))
    out = np.concatenate(outs, axis=0)  # [64, S, D]
    return np.ascontiguousarray(out.reshape(B, H, S, D))


def kernel(Q, K, V):
    in_maps = prepare_in_maps(Q, K, V)
    nc = _get_nc()
    res = run_bass_kernel_spmd(nc, in_maps, core_ids=list(range(N_CORES)))
    return gather_out(res.results)


# revision 20
# speedup vs baseline: 1.0101x; 1.0101x over previous
"""Causal dot-product attention (B=4, H=16, S=2048, D=128) on 8 TRN2 NeuronCores.

Sharding: batch*heads = 64 (b,h) pairs -> 8 heads per core (head parallel, no
communication). Each core runs a flash-attention-style kernel:

  - Host pre-transposes Q,K per head to [D=128, S] in BF16 and packs V per
    head as [kpos=128, kblock, D+1] in bf16 with a ones column at d=128 (the
    PV matmul then produces the softmax denominator for free).
  - Device computes S^T chunks st[k, q] = K_j^T.T @ Q_tile (bf16 matmuls,
    fp32 PSUM), exp(scale*st) on the scalar engine over ~1536-col groups
    (PSUM -> bf16 SBUF), triangular-mask multiplies on diagonal chunks
    (DVE), then PV: acc[q, 0:129] += pt_chunk.T @ V_aug in bf16, accumulated
    over k-blocks in a PSUM bank.
  - Q-tiles are processed in PAIRS with the odd tile's diagonal chunk
    emitted first, so the two 128-wide diagonal chunks sit adjacent in the
    score stream: every 256-wide chunk stays 1KB-aligned inside a PSUM bank
    while the fully-masked half of the diagonal is never computed or exp'd
    (block-causal exact: 17408 of 18432 cols per head).
  - PV/mask/normalize work for group g is emitted interleaved with the QK
    chunks of group g+2: the PE sees QK-MM / PV-MM / PV-MM triples, so every
    LDWEIGHTS hides under a streaming matmul and the in-order PE queue never
    head-of-line blocks on the in-flight exp().
  - Normalize: out = acc[:, :128] * reciprocal(acc[:, 128]) on DVE, written
    bf16 into a per-head staging tile; two output DMAs per head (GpSimd
    queue). Host un-transposes and converts to fp32.

No max-subtraction is needed: scores are ~N(0,1) after the 1/sqrt(128)
scale, so exp() stays in [e-6, e+6] comfortably inside fp32/bf16 range.
"""

import math
import sys
from collections import deque
from contextlib import ExitStack

import numpy as np

for _p in ("/opt/trn_rl_repo", "/root/.axon_site/_ro/trn_rl_repo"):
    if _p not in sys.path:
        sys.path.append(_p)

import ml_dtypes

import concourse.bass as bass
import concourse.tile as tile
from concourse import bacc, mybir
from concourse.bass_utils import run_bass_kernel_spmd

F32 = mybir.dt.float32
BF16 = mybir.dt.bfloat16
AF = mybir.ActivationFunctionType

# Problem constants (hardcoded; kernel.py must be self-contained).
B, H, S, D = 4, 16, 2048, 128
P = 128
N_CORES = 8
NH = (B * H) // N_CORES  # heads per core = 8
SCALE = 1.0 / math.sqrt(128.0)  # D_MODEL = 128

QTW = 256   # q-tile width (two 128-row PSUM subtiles per tile)
GCOLS = 6 * 256  # score-group capacity per exp() (st tile = 3 PSUM banks)
LAG = 2     # groups of deferral between exp() and its PV consumption
DRAIN_K = 3  # deferred-work items emitted after each QK chunk


def build_nc(nh=NH, s=S):
    nkb = s // P   # k-blocks per head = 16
    nqt = s // QTW  # q-tiles per head = 8

    nc = bacc.Bacc("TRN2", target_bir_lowering=False, debug=False,
                   enable_asserts=False)
    qt_d = nc.declare_dram_parameter("qt", [nh, P, s], BF16, isOutput=False).ap()
    kt_d = nc.declare_dram_parameter("kt", [nh, P, s], BF16, isOutput=False).ap()
    v_d = nc.declare_dram_parameter("v", [nh, P, nkb, D + 1], BF16,
                                    isOutput=False).ap()
    # mask = [triu | ones | triu | triu]: col 0:128 is the plain triangular
    # mask; the full 512 pattern covers the (a-last-256, a-diag, b-diag)
    # chunk triple in one DVE multiply.
    mask_d = nc.declare_dram_parameter("mask", [P, 4 * P], BF16,
                                       isOutput=False).ap()
    # Output layout [head, qpos-within-subtile, q-subtile, d] so each head is
    # two DMAs with 2-4 KiB contiguous runs per partition; host un-transposes.
    out_d = nc.declare_dram_parameter("out", [nh, P, nkb, D], BF16,
                                      isOutput=True).ap()

    with tile.TileContext(nc) as tc, ExitStack() as ctx:
        kt_pool = ctx.enter_context(tc.tile_pool(name="kt_pool", bufs=3))
        qt_pool = ctx.enter_context(tc.tile_pool(name="qt_pool", bufs=3))
        v_pool = ctx.enter_context(tc.tile_pool(name="v_pool", bufs=3))
        pt_pool = ctx.enter_context(tc.tile_pool(name="pt_pool", bufs=6))
        stage_pool = ctx.enter_context(tc.tile_pool(name="stage_pool", bufs=3))
        rl_pool = ctx.enter_context(tc.tile_pool(name="rl_pool", bufs=8))
        st_pool = ctx.enter_context(tc.tile_pool(name="st_pool", bufs=2,
                                                 space="PSUM"))
        acc_pool = ctx.enter_context(tc.tile_pool(name="acc_pool", bufs=2,
                                                  space="PSUM"))
        misc = ctx.enter_context(tc.tile_pool(name="misc", bufs=1))

        # Warm the ACT exp table immediately (one-time ~2.7us load) so the
        # first real exp() isn't delayed; operand values are don't-care.
        warm_t = misc.tile([P, 1], F32)
        nc.vector.memset(warm_t[:], 0.0)
        nc.scalar.activation(warm_t[:], warm_t[:], AF.Exp, bias=0.0, scale=1.0)

        mask_t = misc.tile([P, 4 * P], BF16)

        state = {"st": None, "pt": None, "fill": 0, "entries": []}
        pend = deque()      # item-groups not yet eligible (LAG deep)
        eligible = deque()  # flat deferred-work queue, drained between QK MMs

        def emit_item(it):
            kind = it[0]
            if kind == "mask":
                _, pt_t, lo, w, mlo = it
                nc.vector.tensor_mul(pt_t[:, lo:lo + w], pt_t[:, lo:lo + w],
                                     mask_t[:, mlo:mlo + w])
            elif kind == "pv":
                _, pt_t, pos, sI, i, j, acc_t, v_t, st_f, sp_f = it
                nc.tensor.matmul(acc_t[:, sI * 129:(sI + 1) * 129],
                                 lhsT=pt_t[:, pos:pos + P],
                                 rhs=v_t[:, j], start=st_f, stop=sp_f)
            elif kind == "norm":
                _, i, acc_t, stage_t = it
                for sI in range(2):
                    g = 2 * i + sI
                    rl = rl_pool.tile([P, 1], F32, tag="rl", name="rl")
                    nc.vector.reciprocal(
                        rl[:], acc_t[:, sI * 129 + 128:sI * 129 + 129])
                    nc.vector.tensor_scalar_mul(
                        stage_t[:, g * P:(g + 1) * P],
                        acc_t[:, sI * 129:sI * 129 + D], rl[:])
            elif kind == "outdma":
                # Sync-queue HWDGE: much faster than the GpSimd SWDGE path,
                # which costs a ~3.6us drain at the end of the kernel.
                _, h, stage_t, lo, hi = it
                nc.sync.dma_start(out=out_d[h, :, lo:hi],
                                  in_=stage_t[:, lo * D:hi * D])

        def drain(k):
            while k > 0 and eligible:
                emit_item(eligible.popleft())
                k -= 1

        def flush(final=False):
            if state["fill"]:
                w = state["fill"]
                st_t, pt_t = state["st"], state["pt"]
                nc.scalar.activation(pt_t[:, :w], st_t[:, :w], AF.Exp,
                                     bias=0.0, scale=SCALE)
                items = []
                ents = state["entries"]
                skip_mask = set()
                for idx, (pos, wd, eh, i, j, acc_t, v_t, stage_t, full) in \
                        enumerate(ents):
                    even = (i % 2 == 0)
                    diag128 = (wd == P)  # j == 2i+1, subtile 1 only
                    # --- triangular masks ---
                    if full and j == 2 * i + 1:
                        items.append(("mask", pt_t, pos + P, P, 0))
                    elif j == 2 * i and idx not in skip_mask:
                        # last full chunk of tile i: triangle on subtile 0.
                        # Merge with an immediately following a-diag/b-diag
                        # pair (positions pos+256, pos+384) when present.
                        if (idx + 2 < len(ents) and ents[idx + 1][1] == P
                                and ents[idx + 2][1] == P
                                and ents[idx + 1][0] == pos + QTW
                                and ents[idx + 2][0] == pos + QTW + P):
                            items.append(("mask", pt_t, pos, 4 * P, 0))
                            skip_mask.add(idx + 1)
                            skip_mask.add(idx + 2)
                        else:
                            items.append(("mask", pt_t, pos, P, 0))
                    elif diag128 and idx not in skip_mask:
                        items.append(("mask", pt_t, pos, P, 0))
                    # --- PV matmuls ---
                    if full:
                        # startup pair: plain q-major order, both tiles start
                        # at (j=0, sI=0) and stop at (j=2i+1, sI=1).
                        for sI in range(2):
                            if j > 2 * i + sI:
                                continue  # fully-masked subtile
                            items.append(("pv", pt_t, pos + sI * P, sI, i, j,
                                          acc_t, v_t, j == 0 and sI == 0,
                                          j == 2 * i + 1))
                        if j == 2 * i + 1:
                            items.append(("norm", i, acc_t, stage_t))
                    elif diag128:
                        st_f = not even          # odd tile: emitted first
                        sp_f = even              # even tile: emitted last
                        items.append(("pv", pt_t, pos, 1, i, j, acc_t, v_t,
                                      st_f, sp_f))
                        if even:
                            items.append(("norm", i, acc_t, stage_t))
                    else:
                        for sI in range(2):
                            st_f = even and j == 0 and sI == 0
                            sp_f = (not even) and j == 2 * i and sI == 1
                            items.append(("pv", pt_t, pos + sI * P, sI, i, j,
                                          acc_t, v_t, st_f, sp_f))
                        if (not even) and j == 2 * i:
                            items.append(("norm", i, acc_t, stage_t))
                    if items and items[-1][0] == "norm":
                        # stream the output out as subtiles complete; the
                        # final piece is tiny so the kernel tail is short.
                        if i == 3:
                            items.append(("outdma", eh, stage_t, 0, 8))
                        elif i == 6:
                            items.append(("outdma", eh, stage_t, 8, 14))
                        elif i == 7:
                            items.append(("outdma", eh, stage_t, 14, nkb))
                pend.append(items)
                lag = 1 if state.get("taillag") else LAG
                while len(pend) > lag:
                    eligible.extend(pend.popleft())
                state.update(st=None, pt=None, fill=0, entries=[])
            if final:
                while pend:
                    eligible.extend(pend.popleft())
                while eligible:
                    emit_item(eligible.popleft())

        stash = {}

        def start_head(h, fine=False):
            kt_t = kt_pool.tile([P, s], BF16, tag="kt", name="kt_t")
            qt_t = qt_pool.tile([P, s], BF16, tag="qt", name="qt_t")
            v_t = v_pool.tile([P, nkb, D + 1], BF16, tag="v", name="v_t")
            stash[h] = (kt_t, qt_t, v_t)
            if fine:
                # head 0: fine-grained loads so the first groups never wait
                # on a full-head DMA.
                nc.sync.dma_start(out=kt_t[:, 0:256], in_=kt_d[h, :, 0:256])
                nc.sync.dma_start(out=qt_t[:, 0:256], in_=qt_d[h, :, 0:256])
                nc.sync.dma_start(out=kt_t[:, 256:512],
                                  in_=kt_d[h, :, 256:512])
                nc.sync.dma_start(out=qt_t[:, 256:512],
                                  in_=qt_d[h, :, 256:512])
                nc.sync.dma_start(out=v_t[:, :4], in_=v_d[h, :, :4])
                nc.sync.dma_start(out=mask_t[:], in_=mask_d)
                nc.sync.dma_start(out=kt_t[:, 512:1024],
                                  in_=kt_d[h, :, 512:1024])
                nc.sync.dma_start(out=qt_t[:, 512:s], in_=qt_d[h, :, 512:s])
                nc.sync.dma_start(out=kt_t[:, 1024:s], in_=kt_d[h, :, 1024:s])
                nc.sync.dma_start(out=v_t[:, 4:], in_=v_d[h, :, 4:])
            else:
                nc.sync.dma_start(out=kt_t[:], in_=kt_d[h])
                nc.sync.dma_start(out=qt_t[:], in_=qt_d[h])
                nc.sync.dma_start(out=v_t[:], in_=v_d[h])

        def emit_chunk(h, i, j, width, kt_t, qt_t, v_t, acc_t, stage_t,
                       full=False):
            if state["fill"] + width > GCOLS or \
                    (width == QTW and state["fill"] % 512 == 384):
                flush()
            if state["fill"] == 0:
                state["st"] = st_pool.tile([P, GCOLS], F32, tag="st",
                                           name="st_t")
                state["pt"] = pt_pool.tile([P, GCOLS], BF16, tag="pt",
                                           name="pt_t")
            pos = state["fill"]
            qlo = i * QTW if (width == QTW or full) else i * QTW + P
            nc.tensor.matmul(state["st"][:, pos:pos + width],
                             lhsT=kt_t[:, j * P:(j + 1) * P],
                             rhs=qt_t[:, qlo:qlo + width],
                             start=True, stop=True)
            state["entries"].append((pos, width, h, i, j, acc_t, v_t, stage_t,
                                     full))
            state["fill"] += width
            if state["fill"] == GCOLS:
                flush()
            drain(DRAIN_K)

        start_head(0, fine=True)
        if nh > 1:
            start_head(1)
        for h in range(nh):
            kt_t, qt_t, v_t = stash.pop(h)
            stage_t = stage_pool.tile([P, nkb * D], BF16, tag="stage",
                                      name="stage_t")
            for a in range(0, nqt, 2):  # q-tile pairs (a, a+1)
                b = a + 1
                if a == nqt - 2 and h + 2 < nh:
                    start_head(h + 2)
                if a == nqt - 2 and h == nh - 1:
                    state["taillag"] = True  # drain eagerly near the end
                if h == 0 and a == 0:
                    # startup pair: plain full-width chunks so the first
                    # matmuls only need the first two input DMAs (no early
                    # kt[384:512] dependency from the diagonal reorder).
                    for t in (a, b):
                        acc_t = acc_pool.tile([P, 512], F32, tag="acc",
                                              name="acc_t")
                        for j in range(2 * t + 2):
                            emit_chunk(h, t, j, QTW, kt_t, qt_t, v_t, acc_t,
                                       stage_t, full=True)
                    continue
                acc_a = acc_pool.tile([P, 512], F32, tag="acc", name="acc_a")
                for j in range(2 * a + 1):
                    emit_chunk(h, a, j, QTW, kt_t, qt_t, v_t, acc_a, stage_t)
                emit_chunk(h, a, 2 * a + 1, P, kt_t, qt_t, v_t, acc_a, stage_t)
                acc_b = acc_pool.tile([P, 512], F32, tag="acc", name="acc_b")
                emit_chunk(h, b, 2 * b + 1, P, kt_t, qt_t, v_t, acc_b, stage_t)
                for j in range(2 * b + 1):
                    emit_chunk(h, b, j, QTW, kt_t, qt_t, v_t, acc_b, stage_t)
        flush(final=True)
    nc.compile()
    return nc


_NC = None


def _get_nc():
    global _NC
    if _NC is None:
        _NC = build_nc()
    return _NC


def prepare_in_maps(Q, K, V):
    """Shard + lay out full [B,H,S,D] inputs into per-core in_maps."""
    Qf = np.asarray(Q, dtype=np.float32).reshape(B * H, S, D)
    Kf = np.asarray(K, dtype=np.float32).reshape(B * H, S, D)
    Vf = np.asarray(V, dtype=np.float32).reshape(B * H, S, D)
    nkb = S // P
    tri = np.triu(np.ones((P, P), dtype=np.float32))
    ones = np.ones((P, P), dtype=np.float32)
    mask = np.concatenate([tri, ones, tri, tri], axis=1).astype(
        ml_dtypes.bfloat16)
    in_maps = []
    for c in range(N_CORES):
        hs = slice(c * NH, (c + 1) * NH)
        qt = np.ascontiguousarray(
            Qf[hs].transpose(0, 2, 1)).astype(ml_dtypes.bfloat16)
        kt = np.ascontiguousarray(
            Kf[hs].transpose(0, 2, 1)).astype(ml_dtypes.bfloat16)
        # V: [NH, S, D] -> [NH, kblock, kpos, D] -> [NH, kpos, kblock, D]
        vv = Vf[hs].reshape(NH, nkb, P, D).transpose(0, 2, 1, 3)
        v_aug = np.ones((NH, P, nkb, D + 1), dtype=ml_dtypes.bfloat16)
        v_aug[..., :D] = vv.astype(ml_dtypes.bfloat16)
        in_maps.append({"qt": qt, "kt": kt, "v": v_aug, "mask": mask})
    return in_maps


def gather_out(results):
    # out per core: [NH, P, nkb, D] bf16, q = subtile*128 + p
    outs = []
    for r in results:
        o = np.asarray(r["out"]).astype(np.float32)  # [NH, 128, 16, 128]
        outs.append(o.transpose(0, 2, 1, 3).reshape(NH, S, D))
    out = np.concatenate(outs, axis=0)  # [64, S, D]
    return np.ascontiguousarray(out.reshape(B, H, S, D))


def kernel(Q, K, V):
    in_maps = prepare_in_maps(Q, K, V)
    nc = _get_nc()
    res = run_bass_kernel_spmd(nc, in_maps, core_ids=list(range(N_CORES)))
    return gather_out(res.results)
